# revision 14
# baseline (speedup 1.0000x reference)
"""Trainium2 Bass kernel for CanonCausalMultiheadAttn.

Sharding: tensor-parallel over heads across 8 cores (2 q-heads + 1 kv-head
per core), both batches replicated. Attention outputs are exchanged with two
head-split AllToAlls whose 129-row chunks carry the unnormalized PV output
plus the softmax-denominator row; normalization happens on the receiving
core so the attention inner loop has no cross-engine normalize chain.

Per-core pipeline (all shapes hardcoded for B=2, S=2048, D=2048):
  Front-end runs per 512-token chunk (pipelined): QKV proj (bf16 matmul)
  -> canon conv (DVE, 3-col tails carried between chunk tiles) -> squares
  (ACT) -> rmsnorm partition sums on GPSIMD partition_all_reduce -> sqrt
  (ACT) + fast reciprocal (DVE) -> rstd broadcast on GPSIMD
  partition_broadcast -> RoPE (DVE, norm weight & 1/sqrt(dh) folded into
  host cos/sin tables; rstd_q AND rstd_k both folded into the roped Q/K).
  Attention: scores in [Sk, Sq] layout, two 512-col blocks per [128,1024]
  PSUM group; causal mask added via identity-matmul accumulation; ONE
  scale-free exp per group (ACT); denominators via ones-column matmuls;
  transposed PV accumulation; ds/PV of chunk j overlap scores of j+1.
  Dual AllToAll (h=0 fires while h=1 attention computes) with Shared-space
  outputs; receiver normalizes via DVE reciprocal + GPSIMD broadcast.
  Output projection contracts even heads first (overlaps AllToAll #1),
  h0 partials flushed to SBUF, h1 accumulated and added back on DVE.
"""
import sys

sys.path.insert(0, '/opt/trn_rl_repo')

import numpy as np
import ml_dtypes

import concourse.bass as bass
import concourse.bass_isa as bass_isa
import concourse.mybir as mybir
import concourse.tile as tile
from concourse import bacc
from concourse.bass_utils import run_bass_kernel_spmd

F32 = mybir.dt.float32
BF16 = mybir.dt.bfloat16
AF = mybir.ActivationFunctionType
ALU = mybir.AluOpType
RED = bass_isa.ReduceOp

B, S, D = 2, 2048, 2048
NH, NKV, DH = 16, 8, 128
K_CONV = 4
EPS = 1e-6
SCALE = 1.0 / float(np.sqrt(DH))
NEG = -1e9
N_CORES = 8
N_CHUNKS = S // 512
N_SKB = S // 128


def _build():
    nc = bacc.Bacc("TRN2", target_bir_lowering=False, debug=False,
                   num_devices=N_CORES)

    hsT = nc.dram_tensor("hsT", [D, B * S], BF16, kind="ExternalInput")
    wT = nc.dram_tensor("wT", [D, 512], BF16, kind="ExternalInput")
    woT = nc.dram_tensor("woT", [D, D], BF16, kind="ExternalInput")
    cw = nc.dram_tensor("cw", [512, K_CONV], F32, kind="ExternalInput")
    ropeAq = nc.dram_tensor("ropeAq", [DH, S], BF16, kind="ExternalInput")
    ropeBq = nc.dram_tensor("ropeBq", [DH, S], BF16, kind="ExternalInput")
    ropeAk = nc.dram_tensor("ropeAk", [DH, S], BF16, kind="ExternalInput")
    ropeBk = nc.dram_tensor("ropeBk", [DH, S], BF16, kind="ExternalInput")
    maskd = nc.dram_tensor("maskd", [128, 128], BF16, kind="ExternalInput")
    identd = nc.dram_tensor("identd", [128, 128], BF16, kind="ExternalInput")
    out = nc.dram_tensor("out", [512, D], F32, kind="ExternalOutput")

    with tile.TileContext(nc) as tc:
        with tc.tile_pool(name="const", bufs=1) as cpool, \
             tc.tile_pool(name="persist", bufs=1) as pers, \
             tc.tile_pool(name="dram", bufs=1, space="DRAM") as dram:

            # ---- constants (weight tile DMA leads the queue) ----
            wt_sb = pers.tile([128, 16 * 512], BF16, tag="wt", name="wt_sb")
            nc.sync.dma_start(
                wt_sb[:].rearrange("p (k s) -> p k s", s=512),
                wT.ap().rearrange("(k p) s -> p k s", p=128))
            wt = wt_sb[:].rearrange("p (k s) -> p k s", s=512)
            ropes = {}
            for nm in ("Aq", "Bq", "Ak", "Bk"):
                ropes[nm] = cpool.tile([DH, S], BF16, tag=f"rope{nm}",
                                       name=f"rope{nm}")
            mask_sb = cpool.tile([128, 128], BF16, tag="mask")
            nc.sync.dma_start(mask_sb[:], maskd.ap())
            iden_sb = cpool.tile([128, 128], BF16, tag="iden")
            nc.sync.dma_start(iden_sb[:], identd.ap())
            cw_sb = []
            for mt in range(4):
                t = cpool.tile([128, K_CONV], F32, tag=f"cw{mt}", name=f"cw{mt}")
                nc.sync.dma_start(t[:], cw.ap()[128 * mt:128 * mt + 128, :])
                cw_sb.append(t)
            ones_col_f = cpool.tile([128, 1], F32, tag="ocf")
            nc.vector.memset(ones_col_f[:], 1.0)
            ones_col_bf = cpool.tile([128, 1], BF16, tag="ocb")
            nc.scalar.copy(ones_col_bf[:], ones_col_f[:])
            ones_row_f = cpool.tile([1, 128], F32, tag="orf")
            nc.vector.memset(ones_row_f[:], 1.0)
            ones_row_bf = cpool.tile([1, 128], BF16, tag="orb")
            nc.scalar.copy(ones_row_bf[:], ones_row_f[:])
            eps_sb = cpool.tile([1, 1], F32, tag="eps")
            nc.vector.memset(eps_sb[:], EPS)
            s0_sb = []
            for mt in range(4):
                t = cpool.tile([128, 1], F32, tag=f"s0{mt}", name=f"s0{mt}")
                nc.vector.tensor_scalar_add(t[:], cw_sb[mt][:, 0:1], 1.0)
                s0_sb.append(t)

            # persistent per-(b,mt) outputs of the front-end
            roped = {}   # (b, mt<3) -> [128, S] bf16 (q: rstd_q&scale folded,
                         #                             k: rstd_k folded)
            vaug = {}    # b -> [128, N_SKB*128] bf16, V in [Sk, d] blocks
            aout = {}    # h -> [128, 8*512] bf16, gathered attention
            for b in range(B):
                vaug[b] = pers.tile([128, N_SKB * 128], BF16, tag=f"vaug{b}",
                                    name=f"vaug{b}")
                for mt in range(3):
                    roped[(b, mt)] = pers.tile([128, S], BF16,
                                               tag=f"roped{b}{mt}",
                                               name=f"roped{b}{mt}")
            for h in range(2):
                aout[h] = pers.tile([128, 8 * 512], BF16, tag=f"aout{h}",
                                    name=f"aout{h}")

            a2a_in = [dram.tile([1024, 512], BF16, tag=f"a2ain{h}",
                                name=f"a2ain{h}") for h in range(2)]
            a2a_out = [dram.tile([1024, 512], BF16, tag=f"a2aout{h}",
                                 name=f"a2aout{h}") for h in range(2)]

            # ============ front-end: QKV + canon + norm + rope ============
            fw_cm = tc.tile_pool(name="fwork", bufs=1)
            qps_cm = tc.tile_pool(name="qps", bufs=1, space="PSUM")
            nps_cm = tc.tile_pool(name="nps", bufs=1, space="PSUM")
            bps_cm = tc.tile_pool(name="bps", bufs=1, space="PSUM")
            fw = fw_cm.__enter__()
            qps = qps_cm.__enter__()
            nps = nps_cm.__enter__()
            bps = bps_cm.__enter__()

            raw_prev = [None] * 4

            def qkv_chunk(b, n):
                lo = 512 * n
                hp = []
                for half in range(2):
                    hs = fw.tile([128, 8 * 512], BF16, tag="hs", bufs=3,
                                 name="hs")
                    nc.sync.dma_start(
                        hs[:].rearrange("p (k s) -> p k s", s=512),
                        hsT.ap()[1024 * half:1024 * (half + 1),
                                 b * S + lo:b * S + lo + 512]
                        .rearrange("(k p) s -> p k s", p=128))
                    hp.append(hs)
                if b == 0 and n == 0:
                    for nm, t in (("Aq", ropeAq), ("Bq", ropeBq),
                                  ("Ak", ropeAk), ("Bk", ropeBk)):
                        nc.sync.dma_start(ropes[nm][:], t.ap())
                psq = [qps.tile([128, 512], F32, tag=f"qk{mt}", bufs=1,
                                name=f"qk{mt}") for mt in range(4)]
                for half in range(2):
                    hv = hp[half][:].rearrange("p (k s) -> p k s", s=512)
                    for kk in range(8):
                        k = 8 * half + kk
                        for mt in range(4):
                            nc.tensor.matmul(
                                psq[mt][:],
                                wt[:, k, 128 * mt:128 * (mt + 1)],
                                hv[:, kk, :],
                                start=(k == 0), stop=(k == 15))
                raws = []
                for mt in range(4):
                    r = fw.tile([128, 515], BF16, tag=f"raw{mt}", bufs=2,
                                name=f"raw{mt}")
                    if n == 0:
                        nc.vector.memset(r[:, 0:3], 0.0)
                    else:
                        nc.scalar.copy(r[:, 0:3], raw_prev[mt][:, 512:515])
                    nc.scalar.copy(r[:, 3:515], psq[mt][:])
                    raws.append(r)
                    raw_prev[mt] = r
                return raws

            def rest_chunk(b, n, raws):
                lo = 512 * n
                # canon conv (residual folded into s0)
                cts = []
                for mt in range(4):
                    c = fw.tile([128, 512], BF16, tag=f"c{mt}", bufs=3,
                                name=f"c{mt}")
                    nc.vector.tensor_scalar_mul(c[:], raws[mt][:, 3:515],
                                                s0_sb[mt][:])
                    for k in range(1, K_CONV):
                        nc.vector.scalar_tensor_tensor(
                            c[:], raws[mt][:, 3 - k:515 - k],
                            cw_sb[mt][:, k:k + 1], c[:], ALU.mult, ALU.add)
                    cts.append(c)
                # rmsnorm: squares (ACT) -> partition sum (POOL) ->
                # sqrt (ACT) -> reciprocal (DVE) -> broadcast (POOL)
                bcfs = []
                for mt in range(3):
                    sq = fw.tile([128, 512], BF16, tag="sq", bufs=3,
                                 name="sq")
                    nc.scalar.activation(sq[:], cts[mt][:], AF.Square)
                    ssq = nps.tile([1, 512], F32, tag="ssq", bufs=2,
                                   name="ssq")
                    nc.tensor.matmul(ssq[:], ones_col_bf[:], sq[:],
                                     start=True, stop=True)
                    srt = fw.tile([1, 512], F32, tag="srt", bufs=2,
                                  name="srt")
                    nc.scalar.activation(srt[:], ssq[:], AF.Sqrt,
                                         bias=eps_sb[:], scale=1.0 / DH)
                    rr = fw.tile([1, 512], F32, tag="rr", bufs=2, name="rr")
                    nc.vector.reciprocal_approx_fast(rr[:], srt[:])
                    rrb = fw.tile([1, 512], BF16, tag="rrb", bufs=2,
                                  name="rrb")
                    nc.scalar.copy(rrb[:], rr[:])
                    bcp = bps.tile([128, 512], F32, tag="bcp", bufs=2,
                                   name="bcp")
                    nc.tensor.matmul(bcp[:], ones_row_bf[:], rrb[:],
                                     start=True, stop=True)
                    bcf = fw.tile([128, 512], BF16, tag=f"bcf{mt}", bufs=2,
                                  name=f"bcf{mt}")
                    nc.scalar.copy(bcf[:], bcp[:])
                    bcfs.append(bcf)
                # rope at chunk width; rstd folded in for q AND k
                for mt in range(3):
                    is_q = mt < 2
                    c = cts[mt]
                    A_ = ropes["Aq"] if is_q else ropes["Ak"]
                    B_ = ropes["Bq"] if is_q else ropes["Bk"]
                    sh = fw.tile([128, 512], BF16, tag="sh", bufs=2,
                                 name="sh")
                    nc.sync.dma_start(sh[0:64, :], c[64:128, :])
                    nc.sync.dma_start(sh[64:128, :], c[0:64, :])
                    nc.vector.tensor_mul(sh[:], sh[:], B_[:, lo:lo + 512])
                    tm = fw.tile([128, 512], BF16, tag="tm", bufs=2,
                                 name="tm")
                    nc.vector.tensor_mul(tm[:], c[:], A_[:, lo:lo + 512])
                    nc.vector.tensor_add(tm[:], tm[:], sh[:])
                    nc.vector.tensor_mul(roped[(b, mt)][:, lo:lo + 512],
                                         tm[:], bcfs[mt][:])
                # V: transpose into [Sk, d] blocks
                for t in range(4):
                    i = 4 * n + t
                    nc.sync.dma_start_transpose(
                        vaug[b][:, 128 * i:128 * (i + 1)],
                        cts[3][:, 128 * t:128 * (t + 1)])

            pending = None
            for b in range(B):
                for n in range(N_CHUNKS):
                    raws = qkv_chunk(b, n)
                    if pending is not None:
                        rest_chunk(*pending)
                    pending = (b, n, raws)
            rest_chunk(*pending)

            bps_cm.__exit__(None, None, None)
            nps_cm.__exit__(None, None, None)
            qps_cm.__exit__(None, None, None)
            fw_cm.__exit__(None, None, None)

            # ======================= attention =======================
            apool_cm = tc.tile_pool(name="apool", bufs=1)
            opool_cm = tc.tile_pool(name="opool", bufs=1)
            scps_cm = tc.tile_pool(name="scps", bufs=1, space="PSUM")
            atps_cm = tc.tile_pool(name="atps", bufs=1, space="PSUM")
            dsps_cm = tc.tile_pool(name="dsps", bufs=1, space="PSUM")
            bps2_cm = tc.tile_pool(name="bps2", bufs=1, space="PSUM")
            apool = apool_cm.__enter__()
            opool = opool_cm.__enter__()
            scps = scps_cm.__enter__()
            atps = atps_cm.__enter__()
            dsps = dsps_cm.__enter__()
            bps2 = bps2_cm.__enter__()

            def attn_scores(h, b, j, state):
                """Emit score matmuls + exp for chunk j; stash pt tiles."""
                KT = roped[(b, 2)]
                QT = roped[(b, h)]
                ni = 4 * j + 4
                pts = []
                for g in range(ni // 2):
                    ps = scps.tile([128, 1024], F32, tag="sc", bufs=2,
                                   name="ps")
                    for u in range(2):
                        i = 2 * g + u
                        base = 512 * u
                        r = i - 4 * j
                        nc.tensor.matmul(
                            ps[:, base:base + 512],
                            KT[:, 128 * i:128 * (i + 1)],
                            QT[:, 512 * j:512 * (j + 1)],
                            start=True, stop=(r < 0))
                        if r >= 0:
                            off = 128 * r
                            nc.tensor.matmul(
                                ps[:, base + off:base + off + 128],
                                iden_sb[:], mask_sb[:],
                                start=False, stop=True)
                    pt = apool.tile([128, 1024], BF16, tag="p", bufs=14,
                                    name="pt")
                    nc.scalar.activation(pt[:], ps[:], AF.Exp)
                    pts.append(pt)
                state[j] = pts

            def attn_dspv(h, b, j, state):
                """Denominators + PV + staging DMAs for chunk j."""
                va = vaug[b]
                ni = 4 * j + 4
                pts = state.pop(j)
                rd = 4 * b + j
                ds = dsps.tile([1, 512], F32, tag="ds", bufs=1, name="ds")
                for i in range(ni):
                    g, u = divmod(i, 2)
                    off = 128 * max(i - 4 * j, 0)
                    nc.tensor.matmul(
                        ds[:, off:512], ones_col_bf[:],
                        pts[g][:, 512 * u + off:512 * u + 512],
                        start=(i == 0), stop=(i == ni - 1))
                at2 = atps.tile([128, 512], F32, tag="at", bufs=2,
                                name="at2")
                for i in range(ni):
                    g, u = divmod(i, 2)
                    off = 128 * max(i - 4 * j, 0)
                    nc.tensor.matmul(
                        at2[:, off:512], va[:, 128 * i:128 * (i + 1)],
                        pts[g][:, 512 * u + off:512 * u + 512],
                        start=(i == 0), stop=(i == ni - 1))
                rcp = apool.tile([1, 512], F32, tag="rcp", bufs=2,
                                 name="rcp")
                nc.vector.reciprocal_approx_fast(rcp[:], ds[:])
                recb = apool.tile([1, 512], BF16, tag="recb", bufs=2,
                                  name="recb")
                nc.scalar.copy(recb[:], rcp[:])
                return at2, recb

            def attn_norm(h, b, j, at2, recb):
                rd = 4 * b + j
                bcp2 = bps2.tile([128, 512], F32, tag="nb", bufs=1,
                                 name="bcp2")
                nc.tensor.matmul(bcp2[:], ones_row_bf[:], recb[:],
                                 start=True, stop=True)
                bcs = apool.tile([128, 512], BF16, tag="bcs", bufs=2,
                                 name="bcs")
                nc.vector.tensor_scalar_mul(bcs[:], bcp2[:], 1.0)
                abf = apool.tile([128, 512], BF16, tag="abf", bufs=2,
                                 name="abf")
                nc.vector.tensor_mul(abf[:], at2[:], bcs[:])
                nc.sync.dma_start(a2a_in[h][128 * rd:128 * (rd + 1), :],
                                  abf[:])

            def attn_block(h, b):
                state = {}
                attn_scores(h, b, 0, state)
                attn_scores(h, b, 1, state)
                n0 = attn_dspv(h, b, 0, state)
                attn_scores(h, b, 2, state)
                n1 = attn_dspv(h, b, 1, state)
                attn_norm(h, b, 0, *n0)
                attn_scores(h, b, 3, state)
                n2 = attn_dspv(h, b, 2, state)
                attn_norm(h, b, 1, *n1)
                n3 = attn_dspv(h, b, 3, state)
                attn_norm(h, b, 2, *n2)
                attn_norm(h, b, 3, *n3)

            def recv_norm(h):
                """Receiver-side gather; outputs arrive normalized.
                Triggers ride the POOL queue to stay off the sync queue."""
                for k in range(8):
                    nc.gpsimd.dma_start(
                        aout[h][:, 512 * k:512 * (k + 1)],
                        a2a_out[h][128 * k:128 * (k + 1), :])

            attn_block(0, 0)
            attn_block(0, 1)
            attn_block(1, 0)
            # AllToAll #0 fires as soon as its inputs (h=0 both batches)
            # are staged; POOL order: cc0 -> wo h0 prefetch -> recv bc h0
            # -> cc1 -> wo h1 -> recv bc h1.
            nc.gpsimd.collective_compute(
                "AllToAll", ALU.bypass,
                replica_groups=[list(range(N_CORES))],
                ins=[a2a_in[0].opt()], outs=[a2a_out[0].opt()],
                cc_dim="Partition")
            # wo prefetch for h=0 (even-head row blocks of WoT)
            wo_t = {}
            for n in range(4):
                for k in range(8):
                    t = opool.tile([128, 512], BF16, tag="woA", bufs=16,
                                   name="wo_t")
                    nc.gpsimd.dma_start(
                        t[:], woT.ap()[128 * k:128 * (k + 1),
                                       512 * n:512 * (n + 1)])
                    wo_t[(0, n, k)] = t
            recv_norm(0)
            attn_block(1, 1)
            nc.gpsimd.collective_compute(
                "AllToAll", ALU.bypass,
                replica_groups=[list(range(N_CORES))],
                ins=[a2a_in[1].opt()], outs=[a2a_out[1].opt()],
                cc_dim="Partition")
            for n in range(4):
                for k in range(8):
                    t = opool.tile([128, 512], BF16, tag="woB", bufs=12,
                                   name="wo_t2")
                    nc.gpsimd.dma_start(
                        t[:], woT.ap()[1024 + 128 * k:1024 + 128 * (k + 1),
                                       512 * n:512 * (n + 1)])
                    wo_t[(1, n, k)] = t

            bps2_cm.__exit__(None, None, None)
            dsps_cm.__exit__(None, None, None)
            atps_cm.__exit__(None, None, None)
            scps_cm.__exit__(None, None, None)

            # ====================== out projection ====================
            ops_cm = tc.tile_pool(name="ops", bufs=1, space="PSUM")
            ops = ops_cm.__enter__()
            partials = {}
            # h=0 contraction (overlaps AllToAll #1); dn-pair inner loop
            # reuses each stationary av block for two matmuls
            av0 = aout[0][:].rearrange("p (k s) -> p k s", s=512)
            for pair in range(2):
                pso = {(dn, mp): ops.tile([128, 512], F32,
                                          tag=f"o{dn}{mp}", bufs=1,
                                          name=f"o{dn}{mp}")
                       for dn in range(2) for mp in range(4)}
                for k in range(8):
                    for mp in range(4):
                        for dn in range(2):
                            n = 2 * pair + dn
                            nc.tensor.matmul(
                                pso[(dn, mp)][:],
                                av0[:, k, 128 * mp:128 * (mp + 1)],
                                wo_t[(0, n, k)][:], start=(k == 0),
                                stop=(k == 7))
                for dn in range(2):
                    for mp in range(4):
                        pt = opool.tile([128, 512], F32, tag="part",
                                        bufs=16, name="part")
                        nc.vector.tensor_scalar_mul(pt[:],
                                                    pso[(dn, mp)][:], 1.0)
                        partials[(2 * pair + dn, mp)] = pt
            recv_norm(1)
            # h=1 contraction + add partials + store
            av1 = aout[1][:].rearrange("p (k s) -> p k s", s=512)
            for pair in range(2):
                pso = {(dn, mp): ops.tile([128, 512], F32,
                                          tag=f"o{dn}{mp}", bufs=1,
                                          name=f"o{dn}{mp}")
                       for dn in range(2) for mp in range(4)}
                for k in range(8):
                    for mp in range(4):
                        for dn in range(2):
                            n = 2 * pair + dn
                            nc.tensor.matmul(
                                pso[(dn, mp)][:],
                                av1[:, k, 128 * mp:128 * (mp + 1)],
                                wo_t[(1, n, k)][:], start=(k == 0),
                                stop=(k == 7))
                for dn in range(2):
                    for mp in range(4):
                        n = 2 * pair + dn
                        os_t = opool.tile([128, 512], F32, tag="osb",
                                          bufs=4, name="os_t")
                        nc.vector.tensor_add(os_t[:], pso[(dn, mp)][:],
                                             partials[(n, mp)][:])
                        nc.sync.dma_start(
                            out.ap()[128 * mp:128 * (mp + 1),
                                     512 * n:512 * (n + 1)], os_t[:])

            ops_cm.__exit__(None, None, None)
            opool_cm.__exit__(None, None, None)
            apool_cm.__exit__(None, None, None)

    nc.compile()
    return nc


_NC_CACHE = None


def _get_nc():
    global _NC_CACHE
    if _NC_CACHE is None:
        _NC_CACHE = _build()
    return _NC_CACHE


def _host_prep(inputs):
    hs = np.asarray(inputs["hidden_states"], dtype=np.float32)
    Wq = np.asarray(inputs["Wq"], dtype=np.float32)
    Wk = np.asarray(inputs["Wk"], dtype=np.float32)
    Wv = np.asarray(inputs["Wv"], dtype=np.float32)
    Wo = np.asarray(inputs["Wo"], dtype=np.float32)
    cqw = np.asarray(inputs["canon_q_w"], dtype=np.float32)
    ckw = np.asarray(inputs["canon_k_w"], dtype=np.float32)
    cvw = np.asarray(inputs["canon_v_w"], dtype=np.float32)
    qnw = np.asarray(inputs["q_norm_w"], dtype=np.float32)
    knw = np.asarray(inputs["k_norm_w"], dtype=np.float32)

    bf = ml_dtypes.bfloat16
    hsT = np.ascontiguousarray(
        np.concatenate([hs[0].T, hs[1].T], axis=1)).astype(bf)
    WqT, WkT, WvT = Wq.T, Wk.T, Wv.T
    # Wo^T with even-head (h=0 per core) row-blocks first, then odd
    woT_full = Wo.T
    blocks = woT_full.reshape(16, 128, D)
    woT = np.ascontiguousarray(
        np.concatenate([blocks[0::2], blocks[1::2]], axis=0).reshape(D, D)
    ).astype(bf)

    inv_freq = 1.0 / (10000.0 ** (np.arange(0, DH, 2, dtype=np.float64) / DH))
    freqs = np.arange(S, dtype=np.float64)[:, None] * inv_freq
    emb = np.concatenate([freqs, freqs], axis=-1)
    cosT, sinT = np.cos(emb).T, np.sin(emb).T

    def make_rope(normw, scale):
        A = cosT * normw[:, None] * scale
        wswap = normw[(np.arange(DH) + 64) % DH]
        sign = np.where(np.arange(DH) < 64, -1.0, 1.0)
        Bc = sinT * wswap[:, None] * sign[:, None] * scale
        return (np.ascontiguousarray(A).astype(bf),
                np.ascontiguousarray(Bc).astype(bf))

    Aq, Bq = make_rope(qnw, SCALE)
    Ak, Bk = make_rope(knw, 1.0)

    p = np.arange(128)[:, None]
    f = np.arange(128)[None, :]
    maskd = np.where(p <= f, 0.0, NEG).astype(bf)
    identd = np.eye(128, dtype=np.float32).astype(bf)

    in_maps = []
    for r in range(N_CORES):
        wTc = np.ascontiguousarray(np.concatenate(
            [WqT[:, 256 * r:256 * r + 256],
             WkT[:, 128 * r:128 * r + 128],
             WvT[:, 128 * r:128 * r + 128]], axis=1)).astype(bf)
        cwc = np.ascontiguousarray(np.concatenate(
            [cqw[256 * r:256 * r + 256],
             ckw[128 * r:128 * r + 128],
             cvw[128 * r:128 * r + 128]], axis=0)).astype(np.float32)
        in_maps.append({
            "hsT": hsT, "wT": wTc, "woT": woT, "cw": cwc,
            "ropeAq": Aq, "ropeBq": Bq, "ropeAk": Ak, "ropeBk": Bk,
            "maskd": maskd, "identd": identd,
        })
    return in_maps


def kernel(**inputs):
    nc = _get_nc()
    in_maps = _host_prep(inputs)
    res = run_bass_kernel_spmd(nc, in_maps, core_ids=list(range(N_CORES)))
    full = np.empty((B, S, D), np.float32)
    for r in range(N_CORES):
        full[r // 4, 512 * (r % 4):512 * (r % 4 + 1), :] = res.results[r]["out"]
    return full
